# revision 17
# baseline (speedup 1.0000x reference)
"""Decoder layer (ExpansionNet_v2) kernel.

Contract: kernel(**inputs) takes FULL unsharded inputs (as produced by
setup_inputs()) and returns the FULL output [512, 20, 512] fp32.

Strategy: pure data parallel over the batch (beam) dim across 8 NeuronCores
(64 batch elements per core), weights replicated. Two Bass/Tile NEFFs carry
the heavy matmul work (fp32 PSUM accumulation):

  launch 1: k/v cross projections ([9216,512]@[512,512] x2) in fp8e4m3 with
            DoubleRow perf mode (2x PE rate; weights prescaled x64 to clear
            the fp8 subnormal floor, rescaled during PSUM evacuation), plus
            the five DynamicExpansionBlock projections cond/key/a/b/sel
            ([1280,512]@[512,512] x5) in bf16. ~62% of total FLOPs.
  launch 2: the FeedForward block in bf16 ([1280,512]@[512,2048], ReLU+bias
            fused on ScalarE, [1280,2048]@[2048,512]), ~24% of total FLOPs.

The remaining per-example bmms (z, ca/cb, attention) and normalizations run
on host in fp32. Precision choices measured end-to-end: all-bf16 1.4e-3,
+fp8 k/v 1.7e-3 (attention damps k/v quantization); fp8 for p5 (1.0e-2) or
FF (2.4e-2) approaches/exceeds the 2e-2 gate and is not used. If the device
path is unavailable the kernel falls back to full-host fp32.
"""

import os
import time
import numpy as np

D = 512
H = 8
DK = 64
DFF = 2048
NE = 16
BS = 512
L = 20
ENC = 144
EPS = 1e-4
NCORES = 8
BPC = BS // NCORES      # 64 batch elements per core
T1 = BPC * ENC          # 9216 cross tokens per core
T2 = BPC * L            # 1280 x tokens per core

# device-launch wall times (steady-state) recorded by the last kernel() call
LAST_DEVICE_NS = 0


def _ln(x, g, b):
    m = x.mean(-1, keepdims=True)
    v = ((x - m) ** 2).mean(-1, keepdims=True)
    return (x - m) / np.sqrt(v + EPS) * g + b


def _lin(x, w, b):
    return x @ w.T + b


# ---------------------------------------------------------------------------
# Device kernels (bf16 operands, fp32 accumulate)
# ---------------------------------------------------------------------------

def _bass_mods():
    import sys
    if "/opt/trn_rl_repo" not in sys.path:
        sys.path.insert(0, "/opt/trn_rl_repo")
    import concourse.bass as bass
    import concourse.tile as tile
    import concourse.mybir as mybir
    from concourse import bacc
    return bass, tile, mybir, bacc


def _build_proj_kernel():
    """Launch 1: k = cxT.T@wkT, v = cxT.T@wvT  (9216 tokens),
    p5[:, j] = x2T.T @ w5T[j]  (1280 tokens, j in cond/key/a/b/sel).
    All outputs bf16 token-major; biases added on host."""
    bass, tile, mybir, bacc = _bass_mods()
    bf = mybir.dt.bfloat16

    f8 = mybir.dt.float8e4
    nc = bacc.Bacc("TRN2", target_bir_lowering=False, debug=False)
    cxt_d = nc.dram_tensor("cxt", [D, T1], f8, kind="ExternalInput").ap()
    x2t_d = nc.dram_tensor("x2t", [D, T2], bf, kind="ExternalInput").ap()
    wk_d = nc.dram_tensor("wkt", [D, D], f8, kind="ExternalInput").ap()
    wv_d = nc.dram_tensor("wvt", [D, D], f8, kind="ExternalInput").ap()
    w5_d = nc.dram_tensor("w5t", [D, 5, D], bf, kind="ExternalInput").ap()
    k_d = nc.dram_tensor("k", [T1, D], bf, kind="ExternalOutput").ap()
    v_d = nc.dram_tensor("v", [T1, D], bf, kind="ExternalOutput").ap()
    p5_d = nc.dram_tensor("p5", [T2, 5, D], bf, kind="ExternalOutput").ap()

    GRP = 4          # m-tiles per DMA group (512 tokens)
    with tile.TileContext(nc) as tc:
        with tc.tile_pool(name="wpool", bufs=1) as wpool, \
             tc.tile_pool(name="xin", bufs=4) as xin, \
             tc.tile_pool(name="kvout", bufs=8) as kvout, \
             tc.tile_pool(name="p5out", bufs=3) as p5out, \
             tc.tile_pool(name="ps", bufs=8, space="PSUM") as ps:
            wk_t = wpool.tile([128, 4, D], f8)
            wv_t = wpool.tile([128, 4, D], f8)
            w5_t = wpool.tile([128, 4, 5, D], bf)
            # spread issuance: each engine's DMA stream serializes on that
            # engine, so use different engines for independent transfers
            nc.gpsimd.dma_start(wk_t[:], wk_d.rearrange("(c p) n -> p c n", p=128))
            nc.scalar.dma_start(wv_t[:], wv_d.rearrange("(c p) n -> p c n", p=128))
            nc.gpsimd.dma_start(w5_t[:], w5_d.rearrange("(c p) j n -> p c j n", p=128))

            # k/v over 9216 cross tokens: 18 groups of 512 tokens.
            # Outputs batched per group: one 512 KiB DMA per (group, k/v)
            # instead of four 128 KiB ones (amortizes DMA setup + sem prop).
            for g in range(T1 // (128 * GRP)):
                x_t = xin.tile([128, 4, 128 * GRP], f8, tag="xin")
                nc.sync.dma_start(
                    x_t[:],
                    cxt_d[:, g * 128 * GRP:(g + 1) * 128 * GRP]
                    .rearrange("(c p) m -> p c m", p=128),
                )
                ok_t = kvout.tile([128, GRP, D], bf, tag="kvout")
                ov_t = kvout.tile([128, GRP, D], bf, tag="kvout")
                for mt in range(GRP):
                    for w_t, o_t in ((wk_t, ok_t), (wv_t, ov_t)):
                        acc = ps.tile([128, D], mybir.dt.float32, tag="ps")
                        # fp8 DoubleRow: 2 K-slices per PE cell, K_eff=256
                        # per matmul, 0.5 cyc/row -> 2 matmuls cover D=512.
                        # Weights host-prescaled x64 (fp8e4m3 subnormal
                        # floor); undone in the evacuation below.
                        for c in range(2):
                            nc.tensor.matmul(
                                acc[:],
                                x_t[:, 2 * c:2 * c + 2, mt * 128:(mt + 1) * 128],
                                w_t[:, 2 * c:2 * c + 2, :],
                                start=(c == 0), stop=(c == 1),
                                perf_mode=mybir.MatmulPerfMode.DoubleRow)
                        # evacuate on DVE: ACT carries the v-out DMA stream,
                        # and its strict FIFO would stall PSUM drains (and
                        # PE) behind each 1.7us transfer
                        nc.vector.tensor_scalar_mul(
                            o_t[:, mt, :], acc[:], 1.0 / 64.0)
                s0 = g * 128 * GRP
                nc.gpsimd.dma_start(
                    k_d[s0:s0 + 128 * GRP, :].rearrange("(t p) n -> p t n", p=128),
                    ok_t[:])
                nc.scalar.dma_start(
                    v_d[s0:s0 + 128 * GRP, :].rearrange("(t p) n -> p t n", p=128),
                    ov_t[:])

            # five DE projections over 1280 x tokens: 10 m-tiles, outputs
            # batched per group (~1.25 MiB per DMA)
            for g in range((T2 + 128 * GRP - 1) // (128 * GRP)):
                mts = min(GRP, T2 // 128 - g * GRP)
                x_t = xin.tile([128, 4, 128 * GRP], bf, tag="xin")
                nc.sync.dma_start(
                    x_t[:, :, :128 * mts],
                    x2t_d[:, g * 128 * GRP:g * 128 * GRP + 128 * mts]
                    .rearrange("(c p) m -> p c m", p=128),
                )
                o_t = p5out.tile([128, GRP, 5, D], bf, tag="p5out")
                for mt in range(mts):
                    for j in range(5):
                        acc = ps.tile([128, D], mybir.dt.float32, tag="ps")
                        for c in range(4):
                            nc.tensor.matmul(
                                acc[:], x_t[:, c, mt * 128:(mt + 1) * 128],
                                w5_t[:, c, j, :], start=(c == 0), stop=(c == 3))
                        nc.vector.tensor_copy(o_t[:, mt, j, :], acc[:])
                s0 = g * 128 * GRP
                nc.gpsimd.dma_start(
                    p5_d[s0:s0 + 128 * mts, :, :]
                    .rearrange("(t p) j n -> p t j n", p=128),
                    o_t[:, :mts, :, :])
    nc.compile()
    return nc


def _build_ff_kernel():
    """Launch 2: ff = relu(x3 @ ff1_w.T + b1) @ ff2_w.T (1280 tokens).
    hT [2048, 1280] kept feature-major in SBUF (bf16); b2 added on host."""
    bass, tile, mybir, bacc = _bass_mods()
    bf = mybir.dt.bfloat16
    f32 = mybir.dt.float32

    nc = bacc.Bacc("TRN2", target_bir_lowering=False, debug=False)
    x3t_d = nc.dram_tensor("x3t", [D, T2], bf, kind="ExternalInput").ap()
    w1_d = nc.dram_tensor("w1t", [D, DFF], bf, kind="ExternalInput").ap()   # ff1_w.T
    b1_d = nc.dram_tensor("b1", [128, 16], f32, kind="ExternalInput").ap()
    w2_d = nc.dram_tensor("w2t", [DFF, D], bf, kind="ExternalInput").ap()   # ff2_w.T
    ff_d = nc.dram_tensor("ff", [T2, D], bf, kind="ExternalOutput").ap()

    NTOK = T2            # 1280
    TGS = [(0, 512), (512, 512), (1024, 256)]   # token groups for stage A
    with tile.TileContext(nc) as tc:
        with tc.tile_pool(name="wpool", bufs=1) as wpool, \
             tc.tile_pool(name="hpool", bufs=1) as hpool, \
             tc.tile_pool(name="outp", bufs=4) as outp, \
             tc.tile_pool(name="ps", bufs=6, space="PSUM") as ps:
            w1_t = wpool.tile([128, 4, DFF], bf)
            w2_t = wpool.tile([128, 16, D], bf)
            b1_t = wpool.tile([128, 16], f32)
            x3_t = wpool.tile([128, 4, NTOK], bf)
            # stage-A inputs first, split by K-chunk across engines so the
            # first matmuls start after one chunk lands, not the whole 4 MiB;
            # w2 (stage B only) goes last on its own engine stream
            nc.sync.dma_start(b1_t[:], b1_d)
            for c in range(4):
                nc.sync.dma_start(
                    x3_t[:, c, :],
                    x3t_d[c * 128:(c + 1) * 128, :])
                nc.scalar.dma_start(
                    w1_t[:, c, :],
                    w1_d[c * 128:(c + 1) * 128, :])
            nc.gpsimd.dma_start(w2_t[:], w2_d.rearrange("(c p) n -> p c n", p=128))
            h_t = hpool.tile([128, 16, NTOK], bf)

            # stage A: hT[dchunk] = relu(W1.T[:, dchunk].T @ x3T + b1)
            for t0, tn in TGS:
                for dc in range(16):
                    acc = ps.tile([128, 512], f32, tag="ps")
                    for c in range(4):
                        nc.tensor.matmul(
                            acc[:, :tn], w1_t[:, c, dc * 128:(dc + 1) * 128],
                            x3_t[:, c, t0:t0 + tn], start=(c == 0), stop=(c == 3))
                    nc.scalar.activation(
                        h_t[:, dc, t0:t0 + tn], acc[:, :tn],
                        mybir.ActivationFunctionType.Relu,
                        bias=b1_t[:, dc:dc + 1], scale=1.0)

            # stage B: ff[tt] = hT[:, :, tt].T @ W2.T; outputs batched 4 tiles
            # per DMA (512 KiB each)
            for g in range((NTOK // 128 + 3) // 4):
                tts = min(4, NTOK // 128 - g * 4)
                o_t = outp.tile([128, 4, D], bf, tag="outp")
                for mt in range(tts):
                    tt = g * 4 + mt
                    acc = ps.tile([128, 512], f32, tag="ps")
                    for kc in range(16):
                        nc.tensor.matmul(
                            acc[:], h_t[:, kc, tt * 128:(tt + 1) * 128],
                            w2_t[:, kc, :], start=(kc == 0), stop=(kc == 15))
                    nc.vector.tensor_copy(o_t[:, mt, :], acc[:])
                s0 = g * 512
                nc.sync.dma_start(
                    ff_d[s0:s0 + 128 * tts, :].rearrange("(t p) n -> p t n", p=128),
                    o_t[:, :tts, :])
    nc.compile()
    return nc


_CACHE = {"proj": None, "ff": None, "tried": False}


def _get_kernels():
    if _CACHE["proj"] is None and not _CACHE["tried"]:
        _CACHE["tried"] = True
        try:
            _CACHE["proj"] = _build_proj_kernel()
            _CACHE["ff"] = _build_ff_kernel()
        except Exception:
            _CACHE["proj"] = _CACHE["ff"] = None
    return _CACHE["proj"], _CACHE["ff"]


def _run_spmd(nc, in_maps):
    import sys
    if "/opt/trn_rl_repo" not in sys.path:
        sys.path.insert(0, "/opt/trn_rl_repo")
    from concourse import bass_utils
    global LAST_DEVICE_NS
    t0 = time.time()
    res = bass_utils.run_bass_kernel_spmd(nc, in_maps, core_ids=list(range(NCORES)))
    LAST_DEVICE_NS += int((time.time() - t0) * 1e9)
    return res.results


def kernel(x, cross_x, n_indexes, fw_mask, bw_mask, cross_mask,
           ln1_g, ln1_b, ln2_g, ln2_b, ln3_g, ln3_b,
           de_cond_w, de_cond_b, de_qexp, de_bexp, de_key_w, de_key_b,
           de_a_w, de_a_b, de_b_w, de_b_b, de_sel_w, de_sel_b,
           wq, wq_b, wk, wk_b, wv, wv_b, wo, wo_b,
           ff1_w, ff1_b, ff2_w, ff2_b):
    global LAST_DEVICE_NS
    LAST_DEVICE_NS = 0
    f32 = np.float32
    try:
        import ml_dtypes
        BF = ml_dtypes.bfloat16
    except Exception:
        BF = None
    x = np.asarray(x, f32)
    cross_x = np.asarray(cross_x, f32)
    n_indexes = np.asarray(n_indexes)
    g = {k2: np.asarray(v2, f32) for k2, v2 in dict(
        ln1_g=ln1_g, ln1_b=ln1_b, ln2_g=ln2_g, ln2_b=ln2_b,
        ln3_g=ln3_g, ln3_b=ln3_b,
        de_cond_w=de_cond_w, de_cond_b=de_cond_b, de_qexp=de_qexp,
        de_bexp=de_bexp, de_key_w=de_key_w, de_key_b=de_key_b,
        de_a_w=de_a_w, de_a_b=de_a_b, de_b_w=de_b_w, de_b_b=de_b_b,
        de_sel_w=de_sel_w, de_sel_b=de_sel_b,
        wq=wq, wq_b=wq_b, wk=wk, wk_b=wk_b, wv=wv, wv_b=wv_b,
        wo=wo, wo_b=wo_b, ff1_w=ff1_w, ff1_b=ff1_b,
        ff2_w=ff2_w, ff2_b=ff2_b).items()}

    bs, dec_len, _ = x.shape
    use_dev = BF is not None and os.environ.get("KERNEL_NO_DEVICE", "0") != "1"
    proj_nc = ff_nc = None
    if use_dev:
        proj_nc, ff_nc = _get_kernels()

    # ---- LN1 + DynamicExpansionBlock projections ----
    x2 = _ln(x, g["ln1_g"], g["ln1_b"])

    dev_ok = False
    if proj_nc is not None:
        try:
            import ml_dtypes as _mld
            F8 = _mld.float8_e4m3
            # k/v run in fp8e4m3 DoubleRow: weights prescaled x64 to clear
            # the fp8 subnormal floor (undone on device), activations as-is
            wk_f8 = (np.ascontiguousarray(g["wk"].T) * 64.0).astype(F8)
            wv_f8 = (np.ascontiguousarray(g["wv"].T) * 64.0).astype(F8)
            w5_bf = np.stack(
                [g["de_cond_w"].T, g["de_key_w"].T, g["de_a_w"].T,
                 g["de_b_w"].T, g["de_sel_w"].T], axis=1).astype(BF)
            x2_bf = x2.astype(BF)
            cx_f8 = cross_x.astype(F8)
            in_maps = []
            for c in range(NCORES):
                in_maps.append({
                    "cxt": np.ascontiguousarray(
                        cx_f8[c * BPC:(c + 1) * BPC].reshape(T1, D).T),
                    "x2t": np.ascontiguousarray(
                        x2_bf[c * BPC:(c + 1) * BPC].reshape(T2, D).T),
                    "wkt": wk_f8, "wvt": wv_f8, "w5t": w5_bf,
                })
            res = _run_spmd(proj_nc, in_maps)
            k_full = np.concatenate(
                [r["k"].astype(f32).reshape(BPC, ENC, D) for r in res]) + g["wk_b"]
            v_full = np.concatenate(
                [r["v"].astype(f32).reshape(BPC, ENC, D) for r in res]) + g["wv_b"]
            p5 = np.concatenate(
                [r["p5"].astype(f32).reshape(BPC, L, 5, D) for r in res])
            cond = p5[:, :, 0] + g["de_cond_b"]
            xk = p5[:, :, 1] + g["de_key_b"]
            xa = p5[:, :, 2] + g["de_a_b"]
            xb = p5[:, :, 3] + g["de_b_b"]
            sel_in = p5[:, :, 4] + g["de_sel_b"]
            dev_ok = True
        except Exception:
            dev_ok = False
    if not dev_ok:
        k_full = _lin(cross_x, g["wk"], g["wk_b"])
        v_full = _lin(cross_x, g["wv"], g["wv_b"])
        cond = _lin(x2, g["de_cond_w"], g["de_cond_b"])
        xk = _lin(x2, g["de_key_w"], g["de_key_b"])
        xa = _lin(x2, g["de_a_w"], g["de_a_b"])
        xb = _lin(x2, g["de_b_w"], g["de_b_b"])
        sel_in = _lin(x2, g["de_sel_w"], g["de_sel_b"])

    # ---- DynamicExpansionBlock (host bmms) ----
    cond4 = cond[:, :, None, :]
    qe = g["de_qexp"][n_indexes][:, None]
    be = g["de_bexp"][n_indexes][:, None]
    query = (qe + cond4).reshape(bs, dec_len * NE, D)
    bias = (be + cond4).reshape(bs, dec_len * NE, D)
    z = np.einsum("bqd,bkd->bqk", query, xk, optimize=True) / f32(np.sqrt(D))
    fwm = fw_mask != 0
    a_fw = np.where(fwm, np.maximum(z, 0.0), 0.0)
    b_fw = np.where(fwm, np.maximum(-z, 0.0), 0.0)
    a_fw = a_fw / (a_fw.sum(-1, keepdims=True) + EPS)
    b_fw = b_fw / (b_fw.sum(-1, keepdims=True) + EPS)
    ca = a_fw @ xa
    cb = b_fw @ xb
    zt = z.transpose(0, 2, 1)
    bwm = bw_mask != 0
    a_bw = np.where(bwm, np.maximum(zt, 0.0), 0.0)
    b_bw = np.where(bwm, np.maximum(-zt, 0.0), 0.0)
    a_bw = a_bw / (a_bw.sum(-1, keepdims=True) + EPS)
    b_bw = b_bw / (b_bw.sum(-1, keepdims=True) + EPS)
    ca = a_bw @ (ca + bias)
    cb = b_bw @ (cb + bias)
    sel = 1.0 / (1.0 + np.exp(-sel_in))
    x = x + sel * ca + (1.0 - sel) * cb

    # ---- cross MHA (host; k/v from device) ----
    x2 = _ln(x, g["ln2_g"], g["ln2_b"])
    q = _lin(x2, g["wq"], g["wq_b"]).reshape(bs, dec_len, H, DK).transpose(0, 2, 1, 3)
    enc_len = cross_x.shape[1]
    k = k_full.reshape(bs, enc_len, H, DK).transpose(0, 2, 1, 3)
    v = v_full.reshape(bs, enc_len, H, DK).transpose(0, 2, 1, 3)
    s = np.einsum("bhqd,bhkd->bhqk", q, k, optimize=True) / f32(np.sqrt(DK))
    s = np.where(cross_mask[:, :, :, :] == 1, f32(-1000.0), s)
    s = s - s.max(-1, keepdims=True)
    e = np.exp(s)
    att = e / e.sum(-1, keepdims=True)
    o = np.einsum("bhqk,bhkd->bhqd", att, v,
                  optimize=True).transpose(0, 2, 1, 3).reshape(bs, dec_len, D)
    x = x + _lin(o, g["wo"], g["wo_b"])

    # ---- FeedForward ----
    x3 = _ln(x, g["ln3_g"], g["ln3_b"])
    dev_ff = False
    if dev_ok and ff_nc is not None:
        try:
            w1_bf = np.ascontiguousarray(g["ff1_w"].T).astype(BF)
            w2_bf = np.ascontiguousarray(g["ff2_w"].T).astype(BF)
            b1 = np.ascontiguousarray(g["ff1_b"].reshape(16, 128).T)
            x3_bf = x3.astype(BF)
            in_maps = []
            for c in range(NCORES):
                in_maps.append({
                    "x3t": np.ascontiguousarray(
                        x3_bf[c * BPC:(c + 1) * BPC].reshape(T2, D).T),
                    "w1t": w1_bf, "b1": b1, "w2t": w2_bf,
                })
            res = _run_spmd(ff_nc, in_maps)
            ff = np.concatenate(
                [r["ff"].astype(f32).reshape(BPC, L, D) for r in res]) + g["ff2_b"]
            dev_ff = True
        except Exception:
            dev_ff = False
    if not dev_ff:
        h = np.maximum(_lin(x3, g["ff1_w"], g["ff1_b"]), 0.0)
        ff = _lin(h, g["ff2_w"], g["ff2_b"])
    x = x + ff
    return x.astype(np.float32)


# revision 21
# speedup vs baseline: 1.2257x; 1.2257x over previous
"""Decoder layer (ExpansionNet_v2) kernel.

Contract: kernel(**inputs) takes FULL unsharded inputs (as produced by
setup_inputs()) and returns the FULL output [512, 20, 512] fp32.

Strategy: pure data parallel over the batch (beam) dim across 8 NeuronCores
(64 batch elements per core), weights replicated. Two Bass/Tile NEFFs carry
the heavy matmul work (fp32 PSUM accumulation):

  launch 1: k/v cross projections ([9216,512]@[512,512] x2) in fp8e4m3 with
            DoubleRow perf mode (2x PE rate; weights prescaled x64 to clear
            the fp8 subnormal floor, rescaled during PSUM evacuation), plus
            the five DynamicExpansionBlock projections cond/key/a/b/sel
            ([1280,512]@[512,512] x5) in bf16. ~62% of total FLOPs.
  launch 2: the FeedForward block in bf16 ([1280,512]@[512,2048], ReLU+bias
            fused on ScalarE, [1280,2048]@[2048,512]), ~24% of total FLOPs.

The remaining per-example bmms (z, ca/cb, attention) and normalizations run
on host in fp32. Precision choices measured end-to-end: all-bf16 1.4e-3,
+fp8 k/v 1.7e-3 (attention damps k/v quantization); fp8 for p5 (1.0e-2) or
FF (2.4e-2) approaches/exceeds the 2e-2 gate and is not used. If the device
path is unavailable the kernel falls back to full-host fp32.
"""

import os
import time
import numpy as np

D = 512
H = 8
DK = 64
DFF = 2048
NE = 16
BS = 512
L = 20
ENC = 144
EPS = 1e-4
NCORES = 8
BPC = BS // NCORES      # 64 batch elements per core
T1 = BPC * ENC          # 9216 cross tokens per core
T2 = BPC * L            # 1280 x tokens per core

# device-launch wall times (steady-state) recorded by the last kernel() call
LAST_DEVICE_NS = 0


def _ln(x, g, b):
    m = x.mean(-1, keepdims=True)
    v = ((x - m) ** 2).mean(-1, keepdims=True)
    return (x - m) / np.sqrt(v + EPS) * g + b


def _lin(x, w, b):
    return x @ w.T + b


# ---------------------------------------------------------------------------
# Device kernels (bf16 operands, fp32 accumulate)
# ---------------------------------------------------------------------------

def _bass_mods():
    import sys
    if "/opt/trn_rl_repo" not in sys.path:
        sys.path.insert(0, "/opt/trn_rl_repo")
    import concourse.bass as bass
    import concourse.tile as tile
    import concourse.mybir as mybir
    from concourse import bacc
    return bass, tile, mybir, bacc


def _build_proj_kernel():
    """Launch 1: k = cxT.T@wkT, v = cxT.T@wvT  (9216 tokens),
    p5[:, j] = x2T.T @ w5T[j]  (1280 tokens, j in cond/key/a/b/sel).
    All outputs bf16 token-major; biases added on host."""
    bass, tile, mybir, bacc = _bass_mods()
    bf = mybir.dt.bfloat16

    f8 = mybir.dt.float8e4
    nc = bacc.Bacc("TRN2", target_bir_lowering=False, debug=False)
    cxt_d = nc.dram_tensor("cxt", [D, T1], f8, kind="ExternalInput").ap()
    x2t_d = nc.dram_tensor("x2t", [D, T2], bf, kind="ExternalInput").ap()
    wk_d = nc.dram_tensor("wkt", [D, D], f8, kind="ExternalInput").ap()
    wv_d = nc.dram_tensor("wvt", [D, D], f8, kind="ExternalInput").ap()
    w5_d = nc.dram_tensor("w5t", [D, 5, D], bf, kind="ExternalInput").ap()
    # k/v emitted as fp8: halves output DMA; quantization is damped by
    # the attention softmax (measured end-to-end below 3e-3)
    k_d = nc.dram_tensor("k", [T1, D], f8, kind="ExternalOutput").ap()
    v_d = nc.dram_tensor("v", [T1, D], f8, kind="ExternalOutput").ap()
    p5_d = nc.dram_tensor("p5", [T2, 5, D], bf, kind="ExternalOutput").ap()

    GRP = 4          # m-tiles per DMA group (512 tokens)
    with tile.TileContext(nc) as tc:
        with tc.tile_pool(name="wpool", bufs=1) as wpool, \
             tc.tile_pool(name="xin", bufs=4) as xin, \
             tc.tile_pool(name="kvout", bufs=8) as kvout, \
             tc.tile_pool(name="p5out", bufs=3) as p5out, \
             tc.tile_pool(name="ps", bufs=4, space="PSUM") as ps:
            wk_t = wpool.tile([128, 4, D], f8)
            wv_t = wpool.tile([128, 4, D], f8)
            w5_t = wpool.tile([128, 4, 5, D], bf)
            # spread issuance: each engine's DMA stream serializes on that
            # engine, so use different engines for independent transfers
            nc.gpsimd.dma_start(wk_t[:], wk_d.rearrange("(c p) n -> p c n", p=128))
            nc.scalar.dma_start(wv_t[:], wv_d.rearrange("(c p) n -> p c n", p=128))
            nc.gpsimd.dma_start(w5_t[:], w5_d.rearrange("(c p) j n -> p c j n", p=128))

            # k/v over 9216 cross tokens: 18 groups of 512 tokens.
            # Outputs batched per group: one 512 KiB DMA per (group, k/v)
            # instead of four 128 KiB ones (amortizes DMA setup + sem prop).
            for g in range(T1 // (128 * GRP)):
                x_t = xin.tile([128, 4, 128 * GRP], f8, tag="xin")
                nc.sync.dma_start(
                    x_t[:],
                    cxt_d[:, g * 128 * GRP:(g + 1) * 128 * GRP]
                    .rearrange("(c p) m -> p c m", p=128),
                )
                okv_t = kvout.tile([128, 2, GRP, D], f8, tag="kvout")
                for mt in range(GRP):
                    # k and v accumulate into one 2-bank PSUM tile so a
                    # single DVE op evacuates both (halves the per-op
                    # chain latency that was pacing PE)
                    acc = ps.tile([128, 2, D], mybir.dt.float32, tag="ps")
                    for kv, w_t in ((0, wk_t), (1, wv_t)):
                        # fp8 DoubleRow: 2 K-slices per PE cell, K_eff=256
                        # per matmul, 0.5 cyc/row -> 2 matmuls cover D=512.
                        # Weights host-prescaled x64 (fp8e4m3 subnormal
                        # floor); undone in the evacuation below.
                        for c in range(2):
                            nc.tensor.matmul(
                                acc[:, kv, :],
                                x_t[:, 2 * c:2 * c + 2, mt * 128:(mt + 1) * 128],
                                w_t[:, 2 * c:2 * c + 2, :],
                                start=(c == 0), stop=(c == 1),
                                perf_mode=mybir.MatmulPerfMode.DoubleRow)
                    # alternate the drain between DVE and ACT: two
                    # independent evacuation pipelines double the
                    # outstanding PSUM round-trips that pace PE
                    if mt % 2 == 0:
                        nc.vector.tensor_scalar_mul(
                            okv_t[:, :, mt, :], acc[:], 1.0 / 64.0)
                    else:
                        nc.scalar.mul(okv_t[:, :, mt, :], acc[:], 1.0 / 64.0)
                s0 = g * 128 * GRP
                nc.gpsimd.dma_start(
                    k_d[s0:s0 + 128 * GRP, :].rearrange("(t p) n -> p t n", p=128),
                    okv_t[:, 0, :, :])
                nc.scalar.dma_start(
                    v_d[s0:s0 + 128 * GRP, :].rearrange("(t p) n -> p t n", p=128),
                    okv_t[:, 1, :, :])

            # five DE projections over 1280 x tokens: 10 m-tiles, outputs
            # batched per group (~1.25 MiB per DMA)
            for g in range((T2 + 128 * GRP - 1) // (128 * GRP)):
                mts = min(GRP, T2 // 128 - g * GRP)
                x_t = xin.tile([128, 4, 128 * GRP], bf, tag="xin")
                nc.sync.dma_start(
                    x_t[:, :, :128 * mts],
                    x2t_d[:, g * 128 * GRP:g * 128 * GRP + 128 * mts]
                    .rearrange("(c p) m -> p c m", p=128),
                )
                o_t = p5out.tile([128, GRP, 5, D], bf, tag="p5out")
                for mt in range(mts):
                    for j0, jn in ((0, 2), (2, 2), (4, 1)):
                        acc = ps.tile([128, 2, D], mybir.dt.float32, tag="ps")
                        for j in range(j0, j0 + jn):
                            for c in range(4):
                                nc.tensor.matmul(
                                    acc[:, j - j0, :],
                                    x_t[:, c, mt * 128:(mt + 1) * 128],
                                    w5_t[:, c, j, :],
                                    start=(c == 0), stop=(c == 3))
                        if (mt + j0) % 2 == 0:
                            nc.vector.tensor_copy(
                                o_t[:, mt, j0:j0 + jn, :], acc[:, :jn, :])
                        else:
                            nc.scalar.copy(
                                o_t[:, mt, j0:j0 + jn, :], acc[:, :jn, :])
                s0 = g * 128 * GRP
                nc.gpsimd.dma_start(
                    p5_d[s0:s0 + 128 * mts, :, :]
                    .rearrange("(t p) j n -> p t j n", p=128),
                    o_t[:, :mts, :, :])
    nc.compile()
    return nc


def _build_ff_kernel():
    """Launch 2: ff = relu(x3 @ ff1_w.T + b1) @ ff2_w.T (1280 tokens).
    hT [2048, 1280] kept feature-major in SBUF (bf16); b2 added on host."""
    bass, tile, mybir, bacc = _bass_mods()
    bf = mybir.dt.bfloat16
    f32 = mybir.dt.float32

    nc = bacc.Bacc("TRN2", target_bir_lowering=False, debug=False)
    x3t_d = nc.dram_tensor("x3t", [D, T2], bf, kind="ExternalInput").ap()
    w1_d = nc.dram_tensor("w1t", [D, DFF], bf, kind="ExternalInput").ap()   # ff1_w.T
    b1_d = nc.dram_tensor("b1", [128, 16], f32, kind="ExternalInput").ap()
    w2_d = nc.dram_tensor("w2t", [DFF, D], bf, kind="ExternalInput").ap()   # ff2_w.T
    ff_d = nc.dram_tensor("ff", [T2, D], bf, kind="ExternalOutput").ap()

    NTOK = T2            # 1280
    TGS = [(0, 512), (512, 512), (1024, 256)]   # token groups for stage A
    with tile.TileContext(nc) as tc:
        with tc.tile_pool(name="wpool", bufs=1) as wpool, \
             tc.tile_pool(name="hpool", bufs=1) as hpool, \
             tc.tile_pool(name="outp", bufs=4) as outp, \
             tc.tile_pool(name="ps", bufs=6, space="PSUM") as ps:
            w1_t = wpool.tile([128, 4, DFF], bf)
            w2_t = wpool.tile([128, 16, D], bf)
            b1_t = wpool.tile([128, 16], f32)
            x3_t = wpool.tile([128, 4, NTOK], bf)
            # stage-A inputs first, split by K-chunk across engines so the
            # first matmuls start after one chunk lands, not the whole 4 MiB;
            # w2 (stage B only) goes last on its own engine stream
            nc.sync.dma_start(b1_t[:], b1_d)
            for c in range(4):
                nc.sync.dma_start(
                    x3_t[:, c, :],
                    x3t_d[c * 128:(c + 1) * 128, :])
                nc.scalar.dma_start(
                    w1_t[:, c, :],
                    w1_d[c * 128:(c + 1) * 128, :])
            nc.gpsimd.dma_start(w2_t[:], w2_d.rearrange("(c p) n -> p c n", p=128))
            h_t = hpool.tile([128, 16, NTOK], bf)

            # stage A: hT[dchunk] = relu(W1.T[:, dchunk].T @ x3T + b1)
            for t0, tn in TGS:
                for dc in range(16):
                    acc = ps.tile([128, 512], f32, tag="ps")
                    for c in range(4):
                        nc.tensor.matmul(
                            acc[:, :tn], w1_t[:, c, dc * 128:(dc + 1) * 128],
                            x3_t[:, c, t0:t0 + tn], start=(c == 0), stop=(c == 3))
                    nc.scalar.activation(
                        h_t[:, dc, t0:t0 + tn], acc[:, :tn],
                        mybir.ActivationFunctionType.Relu,
                        bias=b1_t[:, dc:dc + 1], scale=1.0)

            # stage B: ff[tt] = hT[:, :, tt].T @ W2.T; outputs batched 4 tiles
            # per DMA (512 KiB each)
            for g in range((NTOK // 128 + 3) // 4):
                tts = min(4, NTOK // 128 - g * 4)
                o_t = outp.tile([128, 4, D], bf, tag="outp")
                for mt in range(tts):
                    tt = g * 4 + mt
                    acc = ps.tile([128, 512], f32, tag="ps")
                    for kc in range(16):
                        nc.tensor.matmul(
                            acc[:], h_t[:, kc, tt * 128:(tt + 1) * 128],
                            w2_t[:, kc, :], start=(kc == 0), stop=(kc == 15))
                    nc.vector.tensor_copy(o_t[:, mt, :], acc[:])
                s0 = g * 512
                nc.sync.dma_start(
                    ff_d[s0:s0 + 128 * tts, :].rearrange("(t p) n -> p t n", p=128),
                    o_t[:, :tts, :])
    nc.compile()
    return nc


_CACHE = {"proj": None, "ff": None, "tried": False}


def _get_kernels():
    if _CACHE["proj"] is None and not _CACHE["tried"]:
        _CACHE["tried"] = True
        try:
            _CACHE["proj"] = _build_proj_kernel()
            _CACHE["ff"] = _build_ff_kernel()
        except Exception:
            _CACHE["proj"] = _CACHE["ff"] = None
    return _CACHE["proj"], _CACHE["ff"]


def _run_spmd(nc, in_maps):
    import sys
    if "/opt/trn_rl_repo" not in sys.path:
        sys.path.insert(0, "/opt/trn_rl_repo")
    from concourse import bass_utils
    global LAST_DEVICE_NS
    t0 = time.time()
    res = bass_utils.run_bass_kernel_spmd(nc, in_maps, core_ids=list(range(NCORES)))
    LAST_DEVICE_NS += int((time.time() - t0) * 1e9)
    return res.results


def kernel(x, cross_x, n_indexes, fw_mask, bw_mask, cross_mask,
           ln1_g, ln1_b, ln2_g, ln2_b, ln3_g, ln3_b,
           de_cond_w, de_cond_b, de_qexp, de_bexp, de_key_w, de_key_b,
           de_a_w, de_a_b, de_b_w, de_b_b, de_sel_w, de_sel_b,
           wq, wq_b, wk, wk_b, wv, wv_b, wo, wo_b,
           ff1_w, ff1_b, ff2_w, ff2_b):
    global LAST_DEVICE_NS
    LAST_DEVICE_NS = 0
    f32 = np.float32
    try:
        import ml_dtypes
        BF = ml_dtypes.bfloat16
    except Exception:
        BF = None
    x = np.asarray(x, f32)
    cross_x = np.asarray(cross_x, f32)
    n_indexes = np.asarray(n_indexes)
    g = {k2: np.asarray(v2, f32) for k2, v2 in dict(
        ln1_g=ln1_g, ln1_b=ln1_b, ln2_g=ln2_g, ln2_b=ln2_b,
        ln3_g=ln3_g, ln3_b=ln3_b,
        de_cond_w=de_cond_w, de_cond_b=de_cond_b, de_qexp=de_qexp,
        de_bexp=de_bexp, de_key_w=de_key_w, de_key_b=de_key_b,
        de_a_w=de_a_w, de_a_b=de_a_b, de_b_w=de_b_w, de_b_b=de_b_b,
        de_sel_w=de_sel_w, de_sel_b=de_sel_b,
        wq=wq, wq_b=wq_b, wk=wk, wk_b=wk_b, wv=wv, wv_b=wv_b,
        wo=wo, wo_b=wo_b, ff1_w=ff1_w, ff1_b=ff1_b,
        ff2_w=ff2_w, ff2_b=ff2_b).items()}

    bs, dec_len, _ = x.shape
    use_dev = BF is not None and os.environ.get("KERNEL_NO_DEVICE", "0") != "1"
    proj_nc = ff_nc = None
    if use_dev:
        proj_nc, ff_nc = _get_kernels()

    # ---- LN1 + DynamicExpansionBlock projections ----
    x2 = _ln(x, g["ln1_g"], g["ln1_b"])

    dev_ok = False
    if proj_nc is not None:
        try:
            import ml_dtypes as _mld
            F8 = _mld.float8_e4m3
            # k/v run in fp8e4m3 DoubleRow: weights prescaled x64 to clear
            # the fp8 subnormal floor (undone on device), activations as-is
            wk_f8 = (np.ascontiguousarray(g["wk"].T) * 64.0).astype(F8)
            wv_f8 = (np.ascontiguousarray(g["wv"].T) * 64.0).astype(F8)
            w5_bf = np.stack(
                [g["de_cond_w"].T, g["de_key_w"].T, g["de_a_w"].T,
                 g["de_b_w"].T, g["de_sel_w"].T], axis=1).astype(BF)
            x2_bf = x2.astype(BF)
            cx_f8 = cross_x.astype(F8)
            in_maps = []
            for c in range(NCORES):
                in_maps.append({
                    "cxt": np.ascontiguousarray(
                        cx_f8[c * BPC:(c + 1) * BPC].reshape(T1, D).T),
                    "x2t": np.ascontiguousarray(
                        x2_bf[c * BPC:(c + 1) * BPC].reshape(T2, D).T),
                    "wkt": wk_f8, "wvt": wv_f8, "w5t": w5_bf,
                })
            res = _run_spmd(proj_nc, in_maps)
            k_full = np.concatenate(
                [r["k"].astype(f32).reshape(BPC, ENC, D) for r in res]) + g["wk_b"]
            v_full = np.concatenate(
                [r["v"].astype(f32).reshape(BPC, ENC, D) for r in res]) + g["wv_b"]
            p5 = np.concatenate(
                [r["p5"].astype(f32).reshape(BPC, L, 5, D) for r in res])
            cond = p5[:, :, 0] + g["de_cond_b"]
            xk = p5[:, :, 1] + g["de_key_b"]
            xa = p5[:, :, 2] + g["de_a_b"]
            xb = p5[:, :, 3] + g["de_b_b"]
            sel_in = p5[:, :, 4] + g["de_sel_b"]
            dev_ok = True
        except Exception:
            dev_ok = False
    if not dev_ok:
        k_full = _lin(cross_x, g["wk"], g["wk_b"])
        v_full = _lin(cross_x, g["wv"], g["wv_b"])
        cond = _lin(x2, g["de_cond_w"], g["de_cond_b"])
        xk = _lin(x2, g["de_key_w"], g["de_key_b"])
        xa = _lin(x2, g["de_a_w"], g["de_a_b"])
        xb = _lin(x2, g["de_b_w"], g["de_b_b"])
        sel_in = _lin(x2, g["de_sel_w"], g["de_sel_b"])

    # ---- DynamicExpansionBlock (host bmms) ----
    cond4 = cond[:, :, None, :]
    qe = g["de_qexp"][n_indexes][:, None]
    be = g["de_bexp"][n_indexes][:, None]
    query = (qe + cond4).reshape(bs, dec_len * NE, D)
    bias = (be + cond4).reshape(bs, dec_len * NE, D)
    z = np.einsum("bqd,bkd->bqk", query, xk, optimize=True) / f32(np.sqrt(D))
    fwm = fw_mask != 0
    a_fw = np.where(fwm, np.maximum(z, 0.0), 0.0)
    b_fw = np.where(fwm, np.maximum(-z, 0.0), 0.0)
    a_fw = a_fw / (a_fw.sum(-1, keepdims=True) + EPS)
    b_fw = b_fw / (b_fw.sum(-1, keepdims=True) + EPS)
    ca = a_fw @ xa
    cb = b_fw @ xb
    zt = z.transpose(0, 2, 1)
    bwm = bw_mask != 0
    a_bw = np.where(bwm, np.maximum(zt, 0.0), 0.0)
    b_bw = np.where(bwm, np.maximum(-zt, 0.0), 0.0)
    a_bw = a_bw / (a_bw.sum(-1, keepdims=True) + EPS)
    b_bw = b_bw / (b_bw.sum(-1, keepdims=True) + EPS)
    ca = a_bw @ (ca + bias)
    cb = b_bw @ (cb + bias)
    sel = 1.0 / (1.0 + np.exp(-sel_in))
    x = x + sel * ca + (1.0 - sel) * cb

    # ---- cross MHA (host; k/v from device) ----
    x2 = _ln(x, g["ln2_g"], g["ln2_b"])
    q = _lin(x2, g["wq"], g["wq_b"]).reshape(bs, dec_len, H, DK).transpose(0, 2, 1, 3)
    enc_len = cross_x.shape[1]
    k = k_full.reshape(bs, enc_len, H, DK).transpose(0, 2, 1, 3)
    v = v_full.reshape(bs, enc_len, H, DK).transpose(0, 2, 1, 3)
    s = np.einsum("bhqd,bhkd->bhqk", q, k, optimize=True) / f32(np.sqrt(DK))
    s = np.where(cross_mask[:, :, :, :] == 1, f32(-1000.0), s)
    s = s - s.max(-1, keepdims=True)
    e = np.exp(s)
    att = e / e.sum(-1, keepdims=True)
    o = np.einsum("bhqk,bhkd->bhqd", att, v,
                  optimize=True).transpose(0, 2, 1, 3).reshape(bs, dec_len, D)
    x = x + _lin(o, g["wo"], g["wo_b"])

    # ---- FeedForward ----
    x3 = _ln(x, g["ln3_g"], g["ln3_b"])
    dev_ff = False
    if dev_ok and ff_nc is not None:
        try:
            w1_bf = np.ascontiguousarray(g["ff1_w"].T).astype(BF)
            w2_bf = np.ascontiguousarray(g["ff2_w"].T).astype(BF)
            b1 = np.ascontiguousarray(g["ff1_b"].reshape(16, 128).T)
            x3_bf = x3.astype(BF)
            in_maps = []
            for c in range(NCORES):
                in_maps.append({
                    "x3t": np.ascontiguousarray(
                        x3_bf[c * BPC:(c + 1) * BPC].reshape(T2, D).T),
                    "w1t": w1_bf, "b1": b1, "w2t": w2_bf,
                })
            res = _run_spmd(ff_nc, in_maps)
            ff = np.concatenate(
                [r["ff"].astype(f32).reshape(BPC, L, D) for r in res]) + g["ff2_b"]
            dev_ff = True
        except Exception:
            dev_ff = False
    if not dev_ff:
        h = np.maximum(_lin(x3, g["ff1_w"], g["ff1_b"]), 0.0)
        ff = _lin(h, g["ff2_w"], g["ff2_b"])
    x = x + ff
    return x.astype(np.float32)


# revision 26
# speedup vs baseline: 1.2687x; 1.0351x over previous
"""Decoder layer (ExpansionNet_v2) kernel.

Contract: kernel(**inputs) takes FULL unsharded inputs (as produced by
setup_inputs()) and returns the FULL output [512, 20, 512] fp32.

Strategy: pure data parallel over the batch (beam) dim across 8 NeuronCores
(64 batch elements per core), weights replicated. Two Bass/Tile NEFFs carry
the heavy matmul work (fp32 PSUM accumulation):

  launch 1: k/v cross projections ([9216,512]@[512,512] x2) in fp8e4m3 with
            DoubleRow perf mode (2x PE rate; weights prescaled x64 to clear
            the fp8 subnormal floor, rescaled during PSUM evacuation), plus
            the five DynamicExpansionBlock projections cond/key/a/b/sel
            ([1280,512]@[512,512] x5) in bf16. ~62% of total FLOPs.
  launch 2: the FeedForward block in bf16 ([1280,512]@[512,2048], ReLU+bias
            fused on ScalarE, [1280,2048]@[2048,512]), ~24% of total FLOPs.

The remaining per-example bmms (z, ca/cb, attention) and normalizations run
on host in fp32. Precision choices measured end-to-end: all-bf16 1.4e-3,
+fp8 k/v 1.7e-3 (attention damps k/v quantization); fp8 for p5 (1.0e-2) or
FF (2.4e-2) approaches/exceeds the 2e-2 gate and is not used. If the device
path is unavailable the kernel falls back to full-host fp32.
"""

import os
import time
import numpy as np

D = 512
H = 8
DK = 64
DFF = 2048
NE = 16
BS = 512
L = 20
ENC = 144
EPS = 1e-4
NCORES = 8
BPC = BS // NCORES      # 64 batch elements per core
T1 = BPC * ENC          # 9216 cross tokens per core
T2 = BPC * L            # 1280 x tokens per core

# device-launch wall times (steady-state) recorded by the last kernel() call
LAST_DEVICE_NS = 0


def _ln(x, g, b):
    m = x.mean(-1, keepdims=True)
    v = ((x - m) ** 2).mean(-1, keepdims=True)
    return (x - m) / np.sqrt(v + EPS) * g + b


def _lin(x, w, b):
    return x @ w.T + b


# ---------------------------------------------------------------------------
# Device kernels (bf16 operands, fp32 accumulate)
# ---------------------------------------------------------------------------

def _bass_mods():
    import sys
    if "/opt/trn_rl_repo" not in sys.path:
        sys.path.insert(0, "/opt/trn_rl_repo")
    import concourse.bass as bass
    import concourse.tile as tile
    import concourse.mybir as mybir
    from concourse import bacc
    return bass, tile, mybir, bacc


def _build_proj_kernel():
    """Launch 1: k = cxT.T@wkT, v = cxT.T@wvT  (9216 tokens),
    p5[:, j] = x2T.T @ w5T[j]  (1280 tokens, j in cond/key/a/b/sel).
    All outputs bf16 token-major; biases added on host."""
    bass, tile, mybir, bacc = _bass_mods()
    bf = mybir.dt.bfloat16

    f8 = mybir.dt.float8e4
    nc = bacc.Bacc("TRN2", target_bir_lowering=False, debug=False)
    cxt_d = nc.dram_tensor("cxt", [D, T1], f8, kind="ExternalInput").ap()
    x2t_d = nc.dram_tensor("x2t", [D, T2], bf, kind="ExternalInput").ap()
    wk_d = nc.dram_tensor("wkt", [D, D], f8, kind="ExternalInput").ap()
    wv_d = nc.dram_tensor("wvt", [D, D], f8, kind="ExternalInput").ap()
    w5_d = nc.dram_tensor("w5t", [D, 5, D], bf, kind="ExternalInput").ap()
    # k/v emitted as fp8: halves output DMA; quantization is damped by
    # the attention softmax (measured end-to-end below 3e-3)
    k_d = nc.dram_tensor("k", [T1, D], f8, kind="ExternalOutput").ap()
    v_d = nc.dram_tensor("v", [T1, D], f8, kind="ExternalOutput").ap()
    p5_d = nc.dram_tensor("p5", [T2, 5, D], bf, kind="ExternalOutput").ap()

    GRP = 4          # m-tiles per DMA group (512 tokens)
    with tile.TileContext(nc) as tc:
        with tc.tile_pool(name="wpool", bufs=1) as wpool, \
             tc.tile_pool(name="xin", bufs=4) as xin, \
             tc.tile_pool(name="kvout", bufs=8) as kvout, \
             tc.tile_pool(name="p5out", bufs=3) as p5out, \
             tc.tile_pool(name="ps", bufs=4, space="PSUM") as ps:
            wk_t = wpool.tile([128, 4, D], f8)
            wv_t = wpool.tile([128, 4, D], f8)
            w5_t = wpool.tile([128, 4, 5, D], bf)
            # spread issuance: each engine's DMA stream serializes on that
            # engine, so use different engines for independent transfers
            nc.gpsimd.dma_start(wk_t[:], wk_d.rearrange("(c p) n -> p c n", p=128))
            nc.scalar.dma_start(wv_t[:], wv_d.rearrange("(c p) n -> p c n", p=128))
            nc.gpsimd.dma_start(w5_t[:], w5_d.rearrange("(c p) j n -> p c j n", p=128))

            # k/v over 9216 cross tokens: 18 groups of 512 tokens.
            # Outputs batched per group: one 512 KiB DMA per (group, k/v)
            # instead of four 128 KiB ones (amortizes DMA setup + sem prop).
            for g in range(T1 // (128 * GRP)):
                x_t = xin.tile([128, 4, 128 * GRP], f8, tag="xin")
                nc.sync.dma_start(
                    x_t[:],
                    cxt_d[:, g * 128 * GRP:(g + 1) * 128 * GRP]
                    .rearrange("(c p) m -> p c m", p=128),
                )
                okv_t = kvout.tile([128, 2, GRP, D], f8, tag="kvout")
                for mt in range(GRP):
                    # k and v accumulate into one 2-bank PSUM tile so a
                    # single DVE op evacuates both (halves the per-op
                    # chain latency that was pacing PE)
                    acc = ps.tile([128, 2, D], mybir.dt.float32, tag="ps")
                    for kv, w_t in ((0, wk_t), (1, wv_t)):
                        # fp8 DoubleRow: 2 K-slices per PE cell, K_eff=256
                        # per matmul, 0.5 cyc/row -> 2 matmuls cover D=512.
                        # Weights host-prescaled x64 (fp8e4m3 subnormal
                        # floor); undone in the evacuation below.
                        for c in range(2):
                            nc.tensor.matmul(
                                acc[:, kv, :],
                                x_t[:, 2 * c:2 * c + 2, mt * 128:(mt + 1) * 128],
                                w_t[:, 2 * c:2 * c + 2, :],
                                start=(c == 0), stop=(c == 1),
                                perf_mode=mybir.MatmulPerfMode.DoubleRow)
                    # alternate the drain between DVE and ACT: two
                    # independent evacuation pipelines double the
                    # outstanding PSUM round-trips that pace PE
                    if mt % 2 == 0:
                        nc.vector.tensor_scalar_mul(
                            okv_t[:, :, mt, :], acc[:], 1.0 / 64.0)
                    else:
                        nc.scalar.mul(okv_t[:, :, mt, :], acc[:], 1.0 / 64.0)
                s0 = g * 128 * GRP
                nc.gpsimd.dma_start(
                    k_d[s0:s0 + 128 * GRP, :].rearrange("(t p) n -> p t n", p=128),
                    okv_t[:, 0, :, :])
                # alternate the v-out stream between ACT and Pool so the
                # transfer blocks each engine's drain FIFO only every
                # other group
                veng = nc.scalar if g % 2 == 0 else nc.gpsimd
                veng.dma_start(
                    v_d[s0:s0 + 128 * GRP, :].rearrange("(t p) n -> p t n", p=128),
                    okv_t[:, 1, :, :])

            # five DE projections over 1280 x tokens: 10 m-tiles, outputs
            # batched per group (~1.25 MiB per DMA)
            for g in range((T2 + 128 * GRP - 1) // (128 * GRP)):
                mts = min(GRP, T2 // 128 - g * GRP)
                x_t = xin.tile([128, 4, 128 * GRP], bf, tag="xin")
                nc.sync.dma_start(
                    x_t[:, :, :128 * mts],
                    x2t_d[:, g * 128 * GRP:g * 128 * GRP + 128 * mts]
                    .rearrange("(c p) m -> p c m", p=128),
                )
                o_t = p5out.tile([128, GRP, 5, D], bf, tag="p5out")
                for mt in range(mts):
                    for j0, jn in ((0, 2), (2, 2), (4, 1)):
                        acc = ps.tile([128, 2, D], mybir.dt.float32, tag="ps")
                        for j in range(j0, j0 + jn):
                            for c in range(4):
                                nc.tensor.matmul(
                                    acc[:, j - j0, :],
                                    x_t[:, c, mt * 128:(mt + 1) * 128],
                                    w5_t[:, c, j, :],
                                    start=(c == 0), stop=(c == 3))
                        if (mt + j0) % 2 == 0:
                            nc.vector.tensor_copy(
                                o_t[:, mt, j0:j0 + jn, :], acc[:, :jn, :])
                        else:
                            nc.scalar.copy(
                                o_t[:, mt, j0:j0 + jn, :], acc[:, :jn, :])
                s0 = g * 128 * GRP
                nc.gpsimd.dma_start(
                    p5_d[s0:s0 + 128 * mts, :, :]
                    .rearrange("(t p) j n -> p t j n", p=128),
                    o_t[:, :mts, :, :])
    nc.compile()
    return nc


def _build_ff_kernel():
    """Launch 2: ff = relu(x3 @ ff1_w.T + b1) @ ff2_w.T (1280 tokens).
    hT [2048, 1280] kept feature-major in SBUF (bf16); b2 added on host."""
    bass, tile, mybir, bacc = _bass_mods()
    bf = mybir.dt.bfloat16
    f32 = mybir.dt.float32

    nc = bacc.Bacc("TRN2", target_bir_lowering=False, debug=False)
    x3t_d = nc.dram_tensor("x3t", [D, T2], bf, kind="ExternalInput").ap()
    w1_d = nc.dram_tensor("w1t", [D, DFF], bf, kind="ExternalInput").ap()   # ff1_w.T
    b1_d = nc.dram_tensor("b1", [128, 16], f32, kind="ExternalInput").ap()
    w2_d = nc.dram_tensor("w2t", [DFF, D], bf, kind="ExternalInput").ap()   # ff2_w.T
    ff_d = nc.dram_tensor("ff", [T2, D], bf, kind="ExternalOutput").ap()

    NTOK = T2            # 1280
    TGS = [(0, 512), (512, 512), (1024, 256)]   # token groups for stage A
    with tile.TileContext(nc) as tc:
        with tc.tile_pool(name="wpool", bufs=1) as wpool, \
             tc.tile_pool(name="hpool", bufs=1) as hpool, \
             tc.tile_pool(name="outp", bufs=4) as outp, \
             tc.tile_pool(name="ps", bufs=6, space="PSUM") as ps:
            w1_t = wpool.tile([128, 4, DFF], bf)
            w2_t = wpool.tile([128, 16, D], bf)
            b1_t = wpool.tile([128, 16], f32)
            x3_t = wpool.tile([128, 4, NTOK], bf)
            # stage-A inputs first, split by K-chunk across engines so the
            # first matmuls start after one chunk lands, not the whole 4 MiB;
            # w2 (stage B only) goes last on its own engine stream
            nc.sync.dma_start(b1_t[:], b1_d)
            for c in range(4):
                nc.sync.dma_start(
                    x3_t[:, c, :],
                    x3t_d[c * 128:(c + 1) * 128, :])
                nc.scalar.dma_start(
                    w1_t[:, c, :],
                    w1_d[c * 128:(c + 1) * 128, :])
            nc.gpsimd.dma_start(w2_t[:], w2_d.rearrange("(c p) n -> p c n", p=128))
            h_t = hpool.tile([128, 16, NTOK], bf)

            # stage A: hT[dchunk] = relu(W1.T[:, dchunk].T @ x3T + b1)
            for t0, tn in TGS:
                for dc in range(16):
                    acc = ps.tile([128, 512], f32, tag="ps")
                    for c in range(4):
                        nc.tensor.matmul(
                            acc[:, :tn], w1_t[:, c, dc * 128:(dc + 1) * 128],
                            x3_t[:, c, t0:t0 + tn], start=(c == 0), stop=(c == 3))
                    nc.scalar.activation(
                        h_t[:, dc, t0:t0 + tn], acc[:, :tn],
                        mybir.ActivationFunctionType.Relu,
                        bias=b1_t[:, dc:dc + 1], scale=1.0)

            # stage B: ff[tt] = hT[:, :, tt].T @ W2.T; outputs batched 4 tiles
            # per DMA (512 KiB each)
            for g in range((NTOK // 128 + 3) // 4):
                tts = min(4, NTOK // 128 - g * 4)
                o_t = outp.tile([128, 4, D], bf, tag="outp")
                for mt in range(tts):
                    tt = g * 4 + mt
                    acc = ps.tile([128, 512], f32, tag="ps")
                    for kc in range(16):
                        nc.tensor.matmul(
                            acc[:], h_t[:, kc, tt * 128:(tt + 1) * 128],
                            w2_t[:, kc, :], start=(kc == 0), stop=(kc == 15))
                    nc.vector.tensor_copy(o_t[:, mt, :], acc[:])
                s0 = g * 512
                nc.sync.dma_start(
                    ff_d[s0:s0 + 128 * tts, :].rearrange("(t p) n -> p t n", p=128),
                    o_t[:, :tts, :])
    nc.compile()
    return nc


_CACHE = {"proj": None, "ff": None, "tried": False}


def _get_kernels():
    if _CACHE["proj"] is None and not _CACHE["tried"]:
        _CACHE["tried"] = True
        try:
            _CACHE["proj"] = _build_proj_kernel()
            _CACHE["ff"] = _build_ff_kernel()
        except Exception:
            _CACHE["proj"] = _CACHE["ff"] = None
    return _CACHE["proj"], _CACHE["ff"]


def _run_spmd(nc, in_maps):
    import sys
    if "/opt/trn_rl_repo" not in sys.path:
        sys.path.insert(0, "/opt/trn_rl_repo")
    from concourse import bass_utils
    global LAST_DEVICE_NS
    t0 = time.time()
    res = bass_utils.run_bass_kernel_spmd(nc, in_maps, core_ids=list(range(NCORES)))
    LAST_DEVICE_NS += int((time.time() - t0) * 1e9)
    return res.results


def kernel(x, cross_x, n_indexes, fw_mask, bw_mask, cross_mask,
           ln1_g, ln1_b, ln2_g, ln2_b, ln3_g, ln3_b,
           de_cond_w, de_cond_b, de_qexp, de_bexp, de_key_w, de_key_b,
           de_a_w, de_a_b, de_b_w, de_b_b, de_sel_w, de_sel_b,
           wq, wq_b, wk, wk_b, wv, wv_b, wo, wo_b,
           ff1_w, ff1_b, ff2_w, ff2_b):
    global LAST_DEVICE_NS
    LAST_DEVICE_NS = 0
    f32 = np.float32
    try:
        import ml_dtypes
        BF = ml_dtypes.bfloat16
    except Exception:
        BF = None
    x = np.asarray(x, f32)
    cross_x = np.asarray(cross_x, f32)
    n_indexes = np.asarray(n_indexes)
    g = {k2: np.asarray(v2, f32) for k2, v2 in dict(
        ln1_g=ln1_g, ln1_b=ln1_b, ln2_g=ln2_g, ln2_b=ln2_b,
        ln3_g=ln3_g, ln3_b=ln3_b,
        de_cond_w=de_cond_w, de_cond_b=de_cond_b, de_qexp=de_qexp,
        de_bexp=de_bexp, de_key_w=de_key_w, de_key_b=de_key_b,
        de_a_w=de_a_w, de_a_b=de_a_b, de_b_w=de_b_w, de_b_b=de_b_b,
        de_sel_w=de_sel_w, de_sel_b=de_sel_b,
        wq=wq, wq_b=wq_b, wk=wk, wk_b=wk_b, wv=wv, wv_b=wv_b,
        wo=wo, wo_b=wo_b, ff1_w=ff1_w, ff1_b=ff1_b,
        ff2_w=ff2_w, ff2_b=ff2_b).items()}

    bs, dec_len, _ = x.shape
    use_dev = BF is not None and os.environ.get("KERNEL_NO_DEVICE", "0") != "1"
    proj_nc = ff_nc = None
    if use_dev:
        proj_nc, ff_nc = _get_kernels()

    # ---- LN1 + DynamicExpansionBlock projections ----
    x2 = _ln(x, g["ln1_g"], g["ln1_b"])

    dev_ok = False
    if proj_nc is not None:
        try:
            import ml_dtypes as _mld
            F8 = _mld.float8_e4m3
            # k/v run in fp8e4m3 DoubleRow: weights prescaled x64 to clear
            # the fp8 subnormal floor (undone on device), activations as-is
            wk_f8 = (np.ascontiguousarray(g["wk"].T) * 64.0).astype(F8)
            wv_f8 = (np.ascontiguousarray(g["wv"].T) * 64.0).astype(F8)
            w5_bf = np.stack(
                [g["de_cond_w"].T, g["de_key_w"].T, g["de_a_w"].T,
                 g["de_b_w"].T, g["de_sel_w"].T], axis=1).astype(BF)
            x2_bf = x2.astype(BF)
            cx_f8 = cross_x.astype(F8)
            in_maps = []
            for c in range(NCORES):
                in_maps.append({
                    "cxt": np.ascontiguousarray(
                        cx_f8[c * BPC:(c + 1) * BPC].reshape(T1, D).T),
                    "x2t": np.ascontiguousarray(
                        x2_bf[c * BPC:(c + 1) * BPC].reshape(T2, D).T),
                    "wkt": wk_f8, "wvt": wv_f8, "w5t": w5_bf,
                })
            res = _run_spmd(proj_nc, in_maps)
            k_full = np.concatenate(
                [r["k"].astype(f32).reshape(BPC, ENC, D) for r in res]) + g["wk_b"]
            v_full = np.concatenate(
                [r["v"].astype(f32).reshape(BPC, ENC, D) for r in res]) + g["wv_b"]
            p5 = np.concatenate(
                [r["p5"].astype(f32).reshape(BPC, L, 5, D) for r in res])
            cond = p5[:, :, 0] + g["de_cond_b"]
            xk = p5[:, :, 1] + g["de_key_b"]
            xa = p5[:, :, 2] + g["de_a_b"]
            xb = p5[:, :, 3] + g["de_b_b"]
            sel_in = p5[:, :, 4] + g["de_sel_b"]
            dev_ok = True
        except Exception:
            dev_ok = False
    if not dev_ok:
        k_full = _lin(cross_x, g["wk"], g["wk_b"])
        v_full = _lin(cross_x, g["wv"], g["wv_b"])
        cond = _lin(x2, g["de_cond_w"], g["de_cond_b"])
        xk = _lin(x2, g["de_key_w"], g["de_key_b"])
        xa = _lin(x2, g["de_a_w"], g["de_a_b"])
        xb = _lin(x2, g["de_b_w"], g["de_b_b"])
        sel_in = _lin(x2, g["de_sel_w"], g["de_sel_b"])

    # ---- DynamicExpansionBlock (host bmms) ----
    cond4 = cond[:, :, None, :]
    qe = g["de_qexp"][n_indexes][:, None]
    be = g["de_bexp"][n_indexes][:, None]
    query = (qe + cond4).reshape(bs, dec_len * NE, D)
    bias = (be + cond4).reshape(bs, dec_len * NE, D)
    z = np.einsum("bqd,bkd->bqk", query, xk, optimize=True) / f32(np.sqrt(D))
    fwm = fw_mask != 0
    a_fw = np.where(fwm, np.maximum(z, 0.0), 0.0)
    b_fw = np.where(fwm, np.maximum(-z, 0.0), 0.0)
    a_fw = a_fw / (a_fw.sum(-1, keepdims=True) + EPS)
    b_fw = b_fw / (b_fw.sum(-1, keepdims=True) + EPS)
    ca = a_fw @ xa
    cb = b_fw @ xb
    zt = z.transpose(0, 2, 1)
    bwm = bw_mask != 0
    a_bw = np.where(bwm, np.maximum(zt, 0.0), 0.0)
    b_bw = np.where(bwm, np.maximum(-zt, 0.0), 0.0)
    a_bw = a_bw / (a_bw.sum(-1, keepdims=True) + EPS)
    b_bw = b_bw / (b_bw.sum(-1, keepdims=True) + EPS)
    ca = a_bw @ (ca + bias)
    cb = b_bw @ (cb + bias)
    sel = 1.0 / (1.0 + np.exp(-sel_in))
    x = x + sel * ca + (1.0 - sel) * cb

    # ---- cross MHA (host; k/v from device) ----
    x2 = _ln(x, g["ln2_g"], g["ln2_b"])
    q = _lin(x2, g["wq"], g["wq_b"]).reshape(bs, dec_len, H, DK).transpose(0, 2, 1, 3)
    enc_len = cross_x.shape[1]
    k = k_full.reshape(bs, enc_len, H, DK).transpose(0, 2, 1, 3)
    v = v_full.reshape(bs, enc_len, H, DK).transpose(0, 2, 1, 3)
    s = np.einsum("bhqd,bhkd->bhqk", q, k, optimize=True) / f32(np.sqrt(DK))
    s = np.where(cross_mask[:, :, :, :] == 1, f32(-1000.0), s)
    s = s - s.max(-1, keepdims=True)
    e = np.exp(s)
    att = e / e.sum(-1, keepdims=True)
    o = np.einsum("bhqk,bhkd->bhqd", att, v,
                  optimize=True).transpose(0, 2, 1, 3).reshape(bs, dec_len, D)
    x = x + _lin(o, g["wo"], g["wo_b"])

    # ---- FeedForward ----
    x3 = _ln(x, g["ln3_g"], g["ln3_b"])
    dev_ff = False
    if dev_ok and ff_nc is not None:
        try:
            w1_bf = np.ascontiguousarray(g["ff1_w"].T).astype(BF)
            w2_bf = np.ascontiguousarray(g["ff2_w"].T).astype(BF)
            b1 = np.ascontiguousarray(g["ff1_b"].reshape(16, 128).T)
            x3_bf = x3.astype(BF)
            in_maps = []
            for c in range(NCORES):
                in_maps.append({
                    "x3t": np.ascontiguousarray(
                        x3_bf[c * BPC:(c + 1) * BPC].reshape(T2, D).T),
                    "w1t": w1_bf, "b1": b1, "w2t": w2_bf,
                })
            res = _run_spmd(ff_nc, in_maps)
            ff = np.concatenate(
                [r["ff"].astype(f32).reshape(BPC, L, D) for r in res]) + g["ff2_b"]
            dev_ff = True
        except Exception:
            dev_ff = False
    if not dev_ff:
        h = np.maximum(_lin(x3, g["ff1_w"], g["ff1_b"]), 0.0)
        ff = _lin(h, g["ff2_w"], g["ff2_b"])
    x = x + ff
    return x.astype(np.float32)


# revision 27
# speedup vs baseline: 1.2811x; 1.0098x over previous
"""Decoder layer (ExpansionNet_v2) kernel.

Contract: kernel(**inputs) takes FULL unsharded inputs (as produced by
setup_inputs()) and returns the FULL output [512, 20, 512] fp32.

Strategy: pure data parallel over the batch (beam) dim across 8 NeuronCores
(64 batch elements per core), weights replicated. Two Bass/Tile NEFFs carry
the heavy matmul work (fp32 PSUM accumulation):

  launch 1: k/v cross projections ([9216,512]@[512,512] x2) in fp8e4m3 with
            DoubleRow perf mode (2x PE rate; weights prescaled x64 to clear
            the fp8 subnormal floor, rescaled during PSUM evacuation), plus
            the five DynamicExpansionBlock projections cond/key/a/b/sel
            ([1280,512]@[512,512] x5) in bf16. ~62% of total FLOPs.
  launch 2: the FeedForward block in bf16 ([1280,512]@[512,2048], ReLU+bias
            fused on ScalarE, [1280,2048]@[2048,512]), ~24% of total FLOPs.

The remaining per-example bmms (z, ca/cb, attention) and normalizations run
on host in fp32. Precision choices measured end-to-end: all-bf16 1.4e-3,
+fp8 k/v 1.7e-3 (attention damps k/v quantization); fp8 for p5 (1.0e-2) or
FF (2.4e-2) approaches/exceeds the 2e-2 gate and is not used. If the device
path is unavailable the kernel falls back to full-host fp32.
"""

import os
import time
import numpy as np

D = 512
H = 8
DK = 64
DFF = 2048
NE = 16
BS = 512
L = 20
ENC = 144
EPS = 1e-4
NCORES = 8
BPC = BS // NCORES      # 64 batch elements per core
T1 = BPC * ENC          # 9216 cross tokens per core
T2 = BPC * L            # 1280 x tokens per core

# device-launch wall times (steady-state) recorded by the last kernel() call
LAST_DEVICE_NS = 0


def _ln(x, g, b):
    m = x.mean(-1, keepdims=True)
    v = ((x - m) ** 2).mean(-1, keepdims=True)
    return (x - m) / np.sqrt(v + EPS) * g + b


def _lin(x, w, b):
    return x @ w.T + b


# ---------------------------------------------------------------------------
# Device kernels (bf16 operands, fp32 accumulate)
# ---------------------------------------------------------------------------

def _bass_mods():
    import sys
    if "/opt/trn_rl_repo" not in sys.path:
        sys.path.insert(0, "/opt/trn_rl_repo")
    import concourse.bass as bass
    import concourse.tile as tile
    import concourse.mybir as mybir
    from concourse import bacc
    return bass, tile, mybir, bacc


def _build_proj_kernel():
    """Launch 1: k = cxT.T@wkT, v = cxT.T@wvT  (9216 tokens),
    p5[:, j] = x2T.T @ w5T[j]  (1280 tokens, j in cond/key/a/b/sel).
    All outputs bf16 token-major; biases added on host."""
    bass, tile, mybir, bacc = _bass_mods()
    bf = mybir.dt.bfloat16

    f8 = mybir.dt.float8e4
    nc = bacc.Bacc("TRN2", target_bir_lowering=False, debug=False)
    cxt_d = nc.dram_tensor("cxt", [D, T1], f8, kind="ExternalInput").ap()
    x2t_d = nc.dram_tensor("x2t", [D, T2], bf, kind="ExternalInput").ap()
    wk_d = nc.dram_tensor("wkt", [D, D], f8, kind="ExternalInput").ap()
    wv_d = nc.dram_tensor("wvt", [D, D], f8, kind="ExternalInput").ap()
    w5_d = nc.dram_tensor("w5t", [D, 5, D], bf, kind="ExternalInput").ap()
    # k/v emitted as fp8: halves output DMA; quantization is damped by
    # the attention softmax (measured end-to-end below 3e-3)
    k_d = nc.dram_tensor("k", [T1, D], f8, kind="ExternalOutput").ap()
    v_d = nc.dram_tensor("v", [T1, D], f8, kind="ExternalOutput").ap()
    p5_d = nc.dram_tensor("p5", [T2, 5, D], bf, kind="ExternalOutput").ap()

    GRP = 4          # m-tiles per DMA group (512 tokens)
    with tile.TileContext(nc) as tc:
        with tc.tile_pool(name="wpool", bufs=1) as wpool, \
             tc.tile_pool(name="xin", bufs=4) as xin, \
             tc.tile_pool(name="kvout", bufs=8) as kvout, \
             tc.tile_pool(name="p5out", bufs=3) as p5out, \
             tc.tile_pool(name="ps", bufs=4, space="PSUM") as ps:
            wk_t = wpool.tile([128, 4, D], f8)
            wv_t = wpool.tile([128, 4, D], f8)
            w5_t = wpool.tile([128, 4, 5, D], bf)
            # spread issuance: each engine's DMA stream serializes on that
            # engine, so use different engines for independent transfers
            nc.gpsimd.dma_start(wk_t[:], wk_d.rearrange("(c p) n -> p c n", p=128))
            nc.scalar.dma_start(wv_t[:], wv_d.rearrange("(c p) n -> p c n", p=128))
            nc.gpsimd.dma_start(w5_t[:], w5_d.rearrange("(c p) j n -> p c j n", p=128))

            # k/v over 9216 cross tokens: 18 groups of 512 tokens.
            # Outputs batched per group: one 512 KiB DMA per (group, k/v)
            # instead of four 128 KiB ones (amortizes DMA setup + sem prop).
            def emit_kv_group(g):
                x_t = xin.tile([128, 4, 128 * GRP], f8, tag="xin")
                nc.sync.dma_start(
                    x_t[:],
                    cxt_d[:, g * 128 * GRP:(g + 1) * 128 * GRP]
                    .rearrange("(c p) m -> p c m", p=128),
                )
                okv_t = kvout.tile([128, 2, GRP, D], f8, tag="kvout")
                for mt in range(GRP):
                    # k and v accumulate into one 2-bank PSUM tile so a
                    # single DVE op evacuates both (halves the per-op
                    # chain latency that was pacing PE)
                    acc = ps.tile([128, 2, D], mybir.dt.float32, tag="ps")
                    for kv, w_t in ((0, wk_t), (1, wv_t)):
                        # fp8 DoubleRow: 2 K-slices per PE cell, K_eff=256
                        # per matmul, 0.5 cyc/row -> 2 matmuls cover D=512.
                        # Weights host-prescaled x64 (fp8e4m3 subnormal
                        # floor); undone in the evacuation below.
                        for c in range(2):
                            nc.tensor.matmul(
                                acc[:, kv, :],
                                x_t[:, 2 * c:2 * c + 2, mt * 128:(mt + 1) * 128],
                                w_t[:, 2 * c:2 * c + 2, :],
                                start=(c == 0), stop=(c == 1),
                                perf_mode=mybir.MatmulPerfMode.DoubleRow)
                    # alternate the drain between DVE and ACT: two
                    # independent evacuation pipelines double the
                    # outstanding PSUM round-trips that pace PE
                    if mt % 2 == 0:
                        nc.vector.tensor_scalar_mul(
                            okv_t[:, :, mt, :], acc[:], 1.0 / 64.0)
                    else:
                        nc.scalar.mul(okv_t[:, :, mt, :], acc[:], 1.0 / 64.0)
                s0 = g * 128 * GRP
                nc.gpsimd.dma_start(
                    k_d[s0:s0 + 128 * GRP, :].rearrange("(t p) n -> p t n", p=128),
                    okv_t[:, 0, :, :])
                # alternate the v-out stream between ACT and Pool so the
                # transfer blocks each engine's drain FIFO only every
                # other group
                veng = nc.scalar if g % 2 == 0 else nc.gpsimd
                veng.dma_start(
                    v_d[s0:s0 + 128 * GRP, :].rearrange("(t p) n -> p t n", p=128),
                    okv_t[:, 1, :, :])

            # five DE projections over 1280 x tokens: 10 m-tiles, outputs
            # batched per group (~1.25 MiB per DMA)
            def emit_p5_group(g):
                mts = min(GRP, T2 // 128 - g * GRP)
                x_t = xin.tile([128, 4, 128 * GRP], bf, tag="xin")
                nc.sync.dma_start(
                    x_t[:, :, :128 * mts],
                    x2t_d[:, g * 128 * GRP:g * 128 * GRP + 128 * mts]
                    .rearrange("(c p) m -> p c m", p=128),
                )
                o_t = p5out.tile([128, GRP, 5, D], bf, tag="p5out")
                for mt in range(mts):
                    for j0, jn in ((0, 2), (2, 2), (4, 1)):
                        acc = ps.tile([128, 2, D], mybir.dt.float32, tag="ps")
                        for j in range(j0, j0 + jn):
                            for c in range(4):
                                nc.tensor.matmul(
                                    acc[:, j - j0, :],
                                    x_t[:, c, mt * 128:(mt + 1) * 128],
                                    w5_t[:, c, j, :],
                                    start=(c == 0), stop=(c == 3))
                        if (mt + j0) % 2 == 0:
                            nc.vector.tensor_copy(
                                o_t[:, mt, j0:j0 + jn, :], acc[:, :jn, :])
                        else:
                            nc.scalar.copy(
                                o_t[:, mt, j0:j0 + jn, :], acc[:, :jn, :])
                s0 = g * 128 * GRP
                nc.gpsimd.dma_start(
                    p5_d[s0:s0 + 128 * mts, :, :]
                    .rearrange("(t p) j n -> p t j n", p=128),
                    o_t[:, :mts, :, :])

            # interleave p5 groups between k/v groups: each section's
            # matmuls fill the other's drain-stall slots
            NKV = T1 // (128 * GRP)
            NP5 = (T2 + 128 * GRP - 1) // (128 * GRP)
            p5_after = {5: 0, 11: 1, 17: 2}
            for g in range(NKV):
                emit_kv_group(g)
                if g in p5_after:
                    emit_p5_group(p5_after[g])
    nc.compile()
    return nc


def _build_ff_kernel():
    """Launch 2: ff = relu(x3 @ ff1_w.T + b1) @ ff2_w.T (1280 tokens).
    hT [2048, 1280] kept feature-major in SBUF (bf16); b2 added on host."""
    bass, tile, mybir, bacc = _bass_mods()
    bf = mybir.dt.bfloat16
    f32 = mybir.dt.float32

    nc = bacc.Bacc("TRN2", target_bir_lowering=False, debug=False)
    x3t_d = nc.dram_tensor("x3t", [D, T2], bf, kind="ExternalInput").ap()
    w1_d = nc.dram_tensor("w1t", [D, DFF], bf, kind="ExternalInput").ap()   # ff1_w.T
    b1_d = nc.dram_tensor("b1", [128, 16], f32, kind="ExternalInput").ap()
    w2_d = nc.dram_tensor("w2t", [DFF, D], bf, kind="ExternalInput").ap()   # ff2_w.T
    ff_d = nc.dram_tensor("ff", [T2, D], bf, kind="ExternalOutput").ap()

    NTOK = T2            # 1280
    TGS = [(0, 512), (512, 512), (1024, 256)]   # token groups for stage A
    with tile.TileContext(nc) as tc:
        with tc.tile_pool(name="wpool", bufs=1) as wpool, \
             tc.tile_pool(name="hpool", bufs=1) as hpool, \
             tc.tile_pool(name="outp", bufs=4) as outp, \
             tc.tile_pool(name="ps", bufs=6, space="PSUM") as ps:
            w1_t = wpool.tile([128, 4, DFF], bf)
            w2_t = wpool.tile([128, 16, D], bf)
            b1_t = wpool.tile([128, 16], f32)
            x3_t = wpool.tile([128, 4, NTOK], bf)
            # stage-A inputs first, split by K-chunk across engines so the
            # first matmuls start after one chunk lands, not the whole 4 MiB;
            # w2 (stage B only) goes last on its own engine stream
            nc.sync.dma_start(b1_t[:], b1_d)
            for c in range(4):
                nc.sync.dma_start(
                    x3_t[:, c, :],
                    x3t_d[c * 128:(c + 1) * 128, :])
                nc.scalar.dma_start(
                    w1_t[:, c, :],
                    w1_d[c * 128:(c + 1) * 128, :])
            nc.gpsimd.dma_start(w2_t[:], w2_d.rearrange("(c p) n -> p c n", p=128))
            h_t = hpool.tile([128, 16, NTOK], bf)

            # stage A: hT[dchunk] = relu(W1.T[:, dchunk].T @ x3T + b1)
            for t0, tn in TGS:
                for dc in range(16):
                    acc = ps.tile([128, 512], f32, tag="ps")
                    for c in range(4):
                        nc.tensor.matmul(
                            acc[:, :tn], w1_t[:, c, dc * 128:(dc + 1) * 128],
                            x3_t[:, c, t0:t0 + tn], start=(c == 0), stop=(c == 3))
                    nc.scalar.activation(
                        h_t[:, dc, t0:t0 + tn], acc[:, :tn],
                        mybir.ActivationFunctionType.Relu,
                        bias=b1_t[:, dc:dc + 1], scale=1.0)

            # stage B: ff[tt] = hT[:, :, tt].T @ W2.T; outputs batched 4 tiles
            # per DMA (512 KiB each)
            for g in range((NTOK // 128 + 3) // 4):
                tts = min(4, NTOK // 128 - g * 4)
                o_t = outp.tile([128, 4, D], bf, tag="outp")
                for mt in range(tts):
                    tt = g * 4 + mt
                    acc = ps.tile([128, 512], f32, tag="ps")
                    for kc in range(16):
                        nc.tensor.matmul(
                            acc[:], h_t[:, kc, tt * 128:(tt + 1) * 128],
                            w2_t[:, kc, :], start=(kc == 0), stop=(kc == 15))
                    nc.vector.tensor_copy(o_t[:, mt, :], acc[:])
                s0 = g * 512
                nc.sync.dma_start(
                    ff_d[s0:s0 + 128 * tts, :].rearrange("(t p) n -> p t n", p=128),
                    o_t[:, :tts, :])
    nc.compile()
    return nc


_CACHE = {"proj": None, "ff": None, "tried": False}


def _get_kernels():
    if _CACHE["proj"] is None and not _CACHE["tried"]:
        _CACHE["tried"] = True
        try:
            _CACHE["proj"] = _build_proj_kernel()
            _CACHE["ff"] = _build_ff_kernel()
        except Exception:
            _CACHE["proj"] = _CACHE["ff"] = None
    return _CACHE["proj"], _CACHE["ff"]


def _run_spmd(nc, in_maps):
    import sys
    if "/opt/trn_rl_repo" not in sys.path:
        sys.path.insert(0, "/opt/trn_rl_repo")
    from concourse import bass_utils
    global LAST_DEVICE_NS
    t0 = time.time()
    res = bass_utils.run_bass_kernel_spmd(nc, in_maps, core_ids=list(range(NCORES)))
    LAST_DEVICE_NS += int((time.time() - t0) * 1e9)
    return res.results


def kernel(x, cross_x, n_indexes, fw_mask, bw_mask, cross_mask,
           ln1_g, ln1_b, ln2_g, ln2_b, ln3_g, ln3_b,
           de_cond_w, de_cond_b, de_qexp, de_bexp, de_key_w, de_key_b,
           de_a_w, de_a_b, de_b_w, de_b_b, de_sel_w, de_sel_b,
           wq, wq_b, wk, wk_b, wv, wv_b, wo, wo_b,
           ff1_w, ff1_b, ff2_w, ff2_b):
    global LAST_DEVICE_NS
    LAST_DEVICE_NS = 0
    f32 = np.float32
    try:
        import ml_dtypes
        BF = ml_dtypes.bfloat16
    except Exception:
        BF = None
    x = np.asarray(x, f32)
    cross_x = np.asarray(cross_x, f32)
    n_indexes = np.asarray(n_indexes)
    g = {k2: np.asarray(v2, f32) for k2, v2 in dict(
        ln1_g=ln1_g, ln1_b=ln1_b, ln2_g=ln2_g, ln2_b=ln2_b,
        ln3_g=ln3_g, ln3_b=ln3_b,
        de_cond_w=de_cond_w, de_cond_b=de_cond_b, de_qexp=de_qexp,
        de_bexp=de_bexp, de_key_w=de_key_w, de_key_b=de_key_b,
        de_a_w=de_a_w, de_a_b=de_a_b, de_b_w=de_b_w, de_b_b=de_b_b,
        de_sel_w=de_sel_w, de_sel_b=de_sel_b,
        wq=wq, wq_b=wq_b, wk=wk, wk_b=wk_b, wv=wv, wv_b=wv_b,
        wo=wo, wo_b=wo_b, ff1_w=ff1_w, ff1_b=ff1_b,
        ff2_w=ff2_w, ff2_b=ff2_b).items()}

    bs, dec_len, _ = x.shape
    use_dev = BF is not None and os.environ.get("KERNEL_NO_DEVICE", "0") != "1"
    proj_nc = ff_nc = None
    if use_dev:
        proj_nc, ff_nc = _get_kernels()

    # ---- LN1 + DynamicExpansionBlock projections ----
    x2 = _ln(x, g["ln1_g"], g["ln1_b"])

    dev_ok = False
    if proj_nc is not None:
        try:
            import ml_dtypes as _mld
            F8 = _mld.float8_e4m3
            # k/v run in fp8e4m3 DoubleRow: weights prescaled x64 to clear
            # the fp8 subnormal floor (undone on device), activations as-is
            wk_f8 = (np.ascontiguousarray(g["wk"].T) * 64.0).astype(F8)
            wv_f8 = (np.ascontiguousarray(g["wv"].T) * 64.0).astype(F8)
            w5_bf = np.stack(
                [g["de_cond_w"].T, g["de_key_w"].T, g["de_a_w"].T,
                 g["de_b_w"].T, g["de_sel_w"].T], axis=1).astype(BF)
            x2_bf = x2.astype(BF)
            cx_f8 = cross_x.astype(F8)
            in_maps = []
            for c in range(NCORES):
                in_maps.append({
                    "cxt": np.ascontiguousarray(
                        cx_f8[c * BPC:(c + 1) * BPC].reshape(T1, D).T),
                    "x2t": np.ascontiguousarray(
                        x2_bf[c * BPC:(c + 1) * BPC].reshape(T2, D).T),
                    "wkt": wk_f8, "wvt": wv_f8, "w5t": w5_bf,
                })
            res = _run_spmd(proj_nc, in_maps)
            k_full = np.concatenate(
                [r["k"].astype(f32).reshape(BPC, ENC, D) for r in res]) + g["wk_b"]
            v_full = np.concatenate(
                [r["v"].astype(f32).reshape(BPC, ENC, D) for r in res]) + g["wv_b"]
            p5 = np.concatenate(
                [r["p5"].astype(f32).reshape(BPC, L, 5, D) for r in res])
            cond = p5[:, :, 0] + g["de_cond_b"]
            xk = p5[:, :, 1] + g["de_key_b"]
            xa = p5[:, :, 2] + g["de_a_b"]
            xb = p5[:, :, 3] + g["de_b_b"]
            sel_in = p5[:, :, 4] + g["de_sel_b"]
            dev_ok = True
        except Exception:
            dev_ok = False
    if not dev_ok:
        k_full = _lin(cross_x, g["wk"], g["wk_b"])
        v_full = _lin(cross_x, g["wv"], g["wv_b"])
        cond = _lin(x2, g["de_cond_w"], g["de_cond_b"])
        xk = _lin(x2, g["de_key_w"], g["de_key_b"])
        xa = _lin(x2, g["de_a_w"], g["de_a_b"])
        xb = _lin(x2, g["de_b_w"], g["de_b_b"])
        sel_in = _lin(x2, g["de_sel_w"], g["de_sel_b"])

    # ---- DynamicExpansionBlock (host bmms) ----
    cond4 = cond[:, :, None, :]
    qe = g["de_qexp"][n_indexes][:, None]
    be = g["de_bexp"][n_indexes][:, None]
    query = (qe + cond4).reshape(bs, dec_len * NE, D)
    bias = (be + cond4).reshape(bs, dec_len * NE, D)
    z = np.einsum("bqd,bkd->bqk", query, xk, optimize=True) / f32(np.sqrt(D))
    fwm = fw_mask != 0
    a_fw = np.where(fwm, np.maximum(z, 0.0), 0.0)
    b_fw = np.where(fwm, np.maximum(-z, 0.0), 0.0)
    a_fw = a_fw / (a_fw.sum(-1, keepdims=True) + EPS)
    b_fw = b_fw / (b_fw.sum(-1, keepdims=True) + EPS)
    ca = a_fw @ xa
    cb = b_fw @ xb
    zt = z.transpose(0, 2, 1)
    bwm = bw_mask != 0
    a_bw = np.where(bwm, np.maximum(zt, 0.0), 0.0)
    b_bw = np.where(bwm, np.maximum(-zt, 0.0), 0.0)
    a_bw = a_bw / (a_bw.sum(-1, keepdims=True) + EPS)
    b_bw = b_bw / (b_bw.sum(-1, keepdims=True) + EPS)
    ca = a_bw @ (ca + bias)
    cb = b_bw @ (cb + bias)
    sel = 1.0 / (1.0 + np.exp(-sel_in))
    x = x + sel * ca + (1.0 - sel) * cb

    # ---- cross MHA (host; k/v from device) ----
    x2 = _ln(x, g["ln2_g"], g["ln2_b"])
    q = _lin(x2, g["wq"], g["wq_b"]).reshape(bs, dec_len, H, DK).transpose(0, 2, 1, 3)
    enc_len = cross_x.shape[1]
    k = k_full.reshape(bs, enc_len, H, DK).transpose(0, 2, 1, 3)
    v = v_full.reshape(bs, enc_len, H, DK).transpose(0, 2, 1, 3)
    s = np.einsum("bhqd,bhkd->bhqk", q, k, optimize=True) / f32(np.sqrt(DK))
    s = np.where(cross_mask[:, :, :, :] == 1, f32(-1000.0), s)
    s = s - s.max(-1, keepdims=True)
    e = np.exp(s)
    att = e / e.sum(-1, keepdims=True)
    o = np.einsum("bhqk,bhkd->bhqd", att, v,
                  optimize=True).transpose(0, 2, 1, 3).reshape(bs, dec_len, D)
    x = x + _lin(o, g["wo"], g["wo_b"])

    # ---- FeedForward ----
    x3 = _ln(x, g["ln3_g"], g["ln3_b"])
    dev_ff = False
    if dev_ok and ff_nc is not None:
        try:
            w1_bf = np.ascontiguousarray(g["ff1_w"].T).astype(BF)
            w2_bf = np.ascontiguousarray(g["ff2_w"].T).astype(BF)
            b1 = np.ascontiguousarray(g["ff1_b"].reshape(16, 128).T)
            x3_bf = x3.astype(BF)
            in_maps = []
            for c in range(NCORES):
                in_maps.append({
                    "x3t": np.ascontiguousarray(
                        x3_bf[c * BPC:(c + 1) * BPC].reshape(T2, D).T),
                    "w1t": w1_bf, "b1": b1, "w2t": w2_bf,
                })
            res = _run_spmd(ff_nc, in_maps)
            ff = np.concatenate(
                [r["ff"].astype(f32).reshape(BPC, L, D) for r in res]) + g["ff2_b"]
            dev_ff = True
        except Exception:
            dev_ff = False
    if not dev_ff:
        h = np.maximum(_lin(x3, g["ff1_w"], g["ff1_b"]), 0.0)
        ff = _lin(h, g["ff2_w"], g["ff2_b"])
    x = x + ff
    return x.astype(np.float32)
